# revision 45
# baseline (speedup 1.0000x reference)
"""Trainium2 Bass kernel for causal multi-head attention (dense transformer).

Problem (hardcoded): x [2, 2048, 1024], 16 heads x 64 dh, causal, fp32 I/O.
Sharding: 8 cores = 2 batches x 4 head-groups. Each core computes 4 heads of
one batch plus a partial output projection [2048, 1024]; the host sums the 4
partials per batch and adds b_O.

On-device everything is computed in transposed orientation (no transposes):
  x^T (host-pretransposed)  ->  Q^T, K^T [dh, s] and V [s, dh] via matmuls
  S^T[k, q] = K Q^T         ->  P^T = exp(S^T / 8) (diag-masked post-exp)
  Z^T[dh, q] = V^T P^T      ->  normalized by column sums (ones-matmul)
  O[s, :]   = (Z^T)^T W_O   (Z^T is directly the lhsT of the O-projection)

Heads are processed in pairs: QK^T packs 2 heads in row-groups (0-63/64-127)
of the PE array, PV packs 2 heads in column-groups -- both run concurrently.

v2 structure (vs v1 baseline):
  - batched input DMAs (host-retiled DRAM layouts, ~10 dma_starts not 60)
  - attention emission interleaved with V/QK1/O projection "filler" matmuls
    at k-group granularity so ScalarE exp runs continuously from ~14us
  - exp: one ACT call per [128,1024] P-tile (trimmed for deep k-groups);
    causal masking via a single [128,128] triangular mask on the diagonal
    blocks only (DVE), masked-out regions are simply never read by PE
  - all PSUM evacuation copies on VectorE, none on ScalarE
  - HAM warmup matmuls during the initial input-DMA wait
  - zero-bias fast path (setup_inputs biases are zeros; general path kept)
"""

import os
from contextlib import ExitStack

import numpy as np

import concourse.tile as tile
from concourse import bacc, mybir
from concourse.bass_utils import run_bass_kernel_spmd

# problem constants
B, S, DM, H, DH = 2, 2048, 1024, 16, 64
P = 128          # partitions
QB = 512         # q block (matmul moving free dim)
NKT = S // P     # 16 k tiles
NQB = S // QB    # 4 q blocks
NDM = DM // P    # 8 d_model tiles
HPC = 4          # heads per core
NCORES = 8

F32 = mybir.dt.float32
BF16 = mybir.dt.bfloat16
F32R = mybir.dt.float32r
FP8 = mybir.dt.float8e4

# fp8 projection path: x and W_Q/W_K/W_V are e4m3 (weights pre-scaled by
# WSCALE to clear the e4m3 subnormal range; the 1/WSCALE^2 folds into the
# softmax exp scale and 1/WSCALE into W_O), matmuls use DoubleRow with a
# 256-wide contraction. Scores/PV stay bf16.
MM_DTYPE = os.environ.get("ATTN_MM_DTYPE", "bf16")
WSCALE = 64.0

_PROGRAM_CACHE = {}
LAST_RESULTS = None  # BassKernelResults of the most recent run (for test.py)


def _mm(nc, out, lhsT, rhs, start, stop, skip=False):
    # skip_group_check: our concurrent groups in one psum bank are
    # partition-disjoint (rows 0-63 vs 64-127).
    return nc.tensor.matmul(
        out, lhsT, rhs, start=start, stop=stop, skip_group_check=skip
    )


def _chain(insts):
    """Ordering-only PE edges so matmuls alternating between row/column
    groups stay adjacent and run concurrently on the array."""
    from concourse.tile import add_dep_helper

    for a, b in zip(insts[1:], insts):
        add_dep_helper(a.ins, b.ins, sync=False, reason="pack-pair order")


def build_program(mm_dtype=MM_DTYPE, with_bias=False):
    """Build the single-core SPMD Bass program (same program on all 8 cores)."""
    key = (mm_dtype, with_bias)
    if key in _PROGRAM_CACHE:
        return _PROGRAM_CACHE[key]

    # HI: projections + scores; LO: P and V (PV matmul)
    HI = {"fp32": F32, "mixed": F32R, "bf16": BF16, "fp8": BF16}[mm_dtype]
    LO = {"fp32": F32, "mixed": BF16, "bf16": BF16, "fp8": BF16}[mm_dtype]
    fp8 = mm_dtype == "fp8"
    PRJ = FP8 if fp8 else HI     # x / W_{Q,K,V} storage + projection matmuls
    NTP = NDM // 2               # fp8: 256-wide contraction pairs

    nc = bacc.Bacc(
        "TRN2", target_bir_lowering=False, debug=False, num_devices=NCORES
    )

    # ---- DRAM I/O (per-core shards, prearranged on host) ----
    # host-retiled so each tensor loads with a single contiguous-AP DMA
    if fp8:
        xT_d = nc.dram_tensor("xT", [P, NTP, 2, S], PRJ, kind="ExternalInput")
        wq_d = nc.dram_tensor(
            "wq", [P, NTP, 2, HPC * DH], PRJ, kind="ExternalInput"
        )
        wk_d = nc.dram_tensor(
            "wk", [P, NTP, 2, HPC * DH], PRJ, kind="ExternalInput"
        )
        wv_d = nc.dram_tensor(
            "wv", [P, NTP, 2, HPC * DH], PRJ, kind="ExternalInput"
        )
    else:
        xT_d = nc.dram_tensor("xT", [P, NDM, S], HI, kind="ExternalInput")
        wq_d = nc.dram_tensor("wq", [P, NDM, HPC * DH], HI, kind="ExternalInput")
        wk_d = nc.dram_tensor("wk", [P, NDM, HPC * DH], HI, kind="ExternalInput")
        wv_d = nc.dram_tensor("wv", [P, NDM, HPC * DH], HI, kind="ExternalInput")
    wo_d = nc.dram_tensor("wo", [P, 2, DM], HI, kind="ExternalInput")
    # masks[:, 0:128] = tri (1 if qc>=kp), masks[:, 128:384] = [zeros | tri]
    masks_d = nc.dram_tensor("masks", [P, 3 * P], LO, kind="ExternalInput")
    if with_bias:
        bq_d = nc.dram_tensor("bq", [2, P], F32, kind="ExternalInput")
        bk_d = nc.dram_tensor("bk", [2, P], F32, kind="ExternalInput")
        bv_d = nc.dram_tensor("bv", [P, HPC * DH], F32, kind="ExternalInput")
    # bf16 partials: host sums 4 per batch in fp64; halves the writeback
    out_d = nc.dram_tensor("out", [S, DM], F32, kind="ExternalOutput")

    with tile.TileContext(nc) as tc, ExitStack() as ctx:
        const = ctx.enter_context(tc.tile_pool(name="const", bufs=1))
        persist = ctx.enter_context(tc.tile_pool(name="persist", bufs=1))

        # ======= PSUM pools: pp(2) + sc(4) + zp(1) + dp(1) = 8 banks =======
        pp = ctx.enter_context(tc.tile_pool(name="pp", bufs=2, space="PSUM"))
        sc = ctx.enter_context(tc.tile_pool(name="sc", bufs=2, space="PSUM"))
        zp = ctx.enter_context(tc.tile_pool(name="zp", bufs=1, space="PSUM"))
        dp = ctx.enter_context(tc.tile_pool(name="dp", bufs=1, space="PSUM"))
        ppool = ctx.enter_context(tc.tile_pool(name="ppool", bufs=10))
        bcpool = ctx.enter_context(tc.tile_pool(name="bcpool", bufs=3))
        ost = ctx.enter_context(tc.tile_pool(name="ost", bufs=4))

        # ---- constants (no DMA deps) ----
        ones64 = const.tile([P, DH], LO, name="ones64", tag="ones64")
        nc.gpsimd.memset(ones64[:], 1.0)
        warm_rhs = const.tile([P, QB], LO, name="warm_rhs", tag="warm")
        nc.gpsimd.memset(warm_rhs[:], 0.0)

        # ---- HAM warmup: keep PE busy during the initial input DMAs ----
        # no data deps; results never read. ~14 x 512-col at the cold clock
        # spans to ~16us, bridging the input-DMA window so the PE never sits
        # idle >3.4us (which would re-throttle HAM to half clock).
        for _ in range(14):
            wtile = dp.tile([P, QB], F32, name="warm", tag="d")
            _mm(nc, wtile[0:DH, :], ones64[:], warm_rhs[:], start=True, stop=True)

        # ---- input DMAs (few, large). sync: critical path; scalar: rest ----
        if fp8:
            xt_sb = persist.tile([P, NTP, 2, S], PRJ, name="xt_sb", tag="xt")
            w_sb = {
                w: persist.tile(
                    [P, NTP, 2, HPC * DH], PRJ, name=f"{w}_sb", tag=w
                )
                for w in ("wq", "wk", "wv")
            }
        else:
            xt_sb = persist.tile([P, NDM, S], HI, name="xt_sb", tag="xt")
            w_sb = {
                w: persist.tile([P, NDM, HPC * DH], HI, name=f"{w}_sb", tag=w)
                for w in ("wq", "wk", "wv")
            }
        wo_sb = persist.tile([P, 2, DM], HI, name="wo_sb", tag="wo")
        masks_sb = const.tile([P, 3 * P], LO, name="masks_sb", tag="masks")
        tri = masks_sb[:, 0:P]

        # split across the two HWDGE queues; first qk chunk needs wk + early
        # xt tiles, so those lead the sync queue
        nxt = NTP if fp8 else NDM
        h = nxt // 2
        nc.sync.dma_start(out=w_sb["wk"][:], in_=wk_d[:])
        nc.scalar.dma_start(out=w_sb["wq"][:], in_=wq_d[:])
        for lo in range(0, h, 2):
            nc.sync.dma_start(
                out=xt_sb[:, lo : lo + 2], in_=xT_d[:, lo : lo + 2]
            )
        for lo in range(h, nxt, 2):
            nc.scalar.dma_start(
                out=xt_sb[:, lo : lo + 2], in_=xT_d[:, lo : lo + 2]
            )
        nc.sync.dma_start(out=w_sb["wv"][:], in_=wv_d[:])
        nc.scalar.dma_start(out=wo_sb[:], in_=wo_d[:])
        nc.sync.dma_start(out=masks_sb[:], in_=masks_d[:])
        if with_bias:
            bq_sb = const.tile([P, 2], F32, name="bq_sb", tag="bq")
            bk_sb = const.tile([P, 2], F32, name="bk_sb", tag="bk")
            for p in range(2):
                nc.scalar.dma_start(out=bq_sb[:, p : p + 1], in_=bq_d[p : p + 1, :])
                nc.scalar.dma_start(out=bk_sb[:, p : p + 1], in_=bk_d[p : p + 1, :])
            bv_sb = const.tile([P, HPC * DH], F32, name="bv_sb", tag="bv")
            nc.scalar.dma_start(out=bv_sb[:], in_=bv_d[:])

        # ---- persistent activations ----
        qt_sb = [
            persist.tile([P, S], HI, name=f"qt{p}", tag=f"qt{p}") for p in range(2)
        ]
        kt_sb = [
            persist.tile([P, S], HI, name=f"kt{p}", tag=f"kt{p}") for p in range(2)
        ]
        # V for all 4 heads: [:, kt, p*128+h*64 : ...] is the PV lhsT slice
        v_all = persist.tile([P, NKT, 2 * P], LO, name="v_all", tag="v")
        zt_sb = [
            persist.tile([P, S], HI, name=f"zt{p}", tag=f"zt{p}") for p in range(2)
        ]

        # ---------------- emission units ----------------

        DR = mybir.MatmulPerfMode.DoubleRow if fp8 else None

        def qk_chunk(p, wname, dst, ch):
            """One Q^T or K^T chunk: [dh-pair 128, QB] for head pair p."""
            qp = pp.tile([P, QB], F32, name="qp", tag="pp")
            if fp8:
                for t in range(NTP):
                    nc.tensor.matmul(
                        qp[:],
                        w_sb[wname][:, t, :, p * P : (p + 1) * P],
                        xt_sb[:, t, :, ch * QB : (ch + 1) * QB],
                        start=(t == 0),
                        stop=(t == NTP - 1),
                        perf_mode=DR,
                    )
            else:
                for t in range(NDM):
                    _mm(
                        nc,
                        qp[:],
                        w_sb[wname][:, t, p * P : (p + 1) * P],
                        xt_sb[:, t, ch * QB : (ch + 1) * QB],
                        start=(t == 0),
                        stop=(t == NDM - 1),
                    )
            dview = dst[:, ch * QB : (ch + 1) * QB]
            if with_bias:
                bias = bq_sb if wname == "wq" else bk_sb
                nc.vector.tensor_scalar_add(dview, qp[:], bias[:, p : p + 1])
            else:
                nc.vector.tensor_copy(dview, qp[:])

        def qk_chunk_pair(p, ch):
            """K and Q chunk ch with t-major interleaved matmuls: during the
            input-DMA window each dm-tile's arrival unblocks TWO matmuls
            instead of one, so the PE fills the DMA shadow better."""
            kp_ = pp.tile([P, QB], F32, name="qp", tag="pp")
            qp_ = pp.tile([P, QB], F32, name="qp", tag="pp")
            nt = NTP if fp8 else NDM
            for t in range(nt):
                for wname, dst in (("wk", kp_), ("wq", qp_)):
                    if fp8:
                        nc.tensor.matmul(
                            dst[:],
                            w_sb[wname][:, t, :, p * P : (p + 1) * P],
                            xt_sb[:, t, :, ch * QB : (ch + 1) * QB],
                            start=(t == 0),
                            stop=(t == nt - 1),
                            perf_mode=DR,
                        )
                    else:
                        _mm(
                            nc,
                            dst[:],
                            w_sb[wname][:, t, p * P : (p + 1) * P],
                            xt_sb[:, t, ch * QB : (ch + 1) * QB],
                            start=(t == 0),
                            stop=(t == nt - 1),
                        )
            for src, dst, bias in (
                (kp_, kt_sb[p], "bk"),
                (qp_, qt_sb[p], "bq"),
            ):
                dview = dst[:, ch * QB : (ch + 1) * QB]
                if with_bias:
                    b = bq_sb if bias == "bq" else bk_sb
                    nc.vector.tensor_scalar_add(dview, src[:], b[:, p : p + 1])
                else:
                    nc.vector.tensor_copy(dview, src[:])

        def v_unit(st):
            """V rows [st*128, (st+1)*128) for all 4 heads."""
            vp = pp.tile([P, QB], F32, name="vp", tag="pp")
            if fp8:
                for t in range(NTP):
                    nc.tensor.matmul(
                        vp[:, 0 : 2 * P],
                        xt_sb[:, t, :, st * P : (st + 1) * P],
                        w_sb["wv"][:, t, :, :],
                        start=(t == 0),
                        stop=(t == NTP - 1),
                        perf_mode=DR,
                    )
            else:
                for t in range(NDM):
                    _mm(
                        nc,
                        vp[:, 0 : 2 * P],
                        xt_sb[:, t, st * P : (st + 1) * P],
                        w_sb["wv"][:, t, :],
                        start=(t == 0),
                        stop=(t == NDM - 1),
                    )
            if with_bias:
                nc.vector.tensor_add(v_all[:, st, :], vp[:, 0 : 2 * P], bv_sb[:])
            else:
                nc.vector.tensor_copy(v_all[:, st, :], vp[:, 0 : 2 * P])

        def o_unit(st, nn):
            """Output projection columns [nn*QB,(nn+1)*QB) for seq tile st."""
            ops = pp.tile([P, QB], F32, name="ops", tag="pp")
            for pr in range(2):
                _mm(
                    nc,
                    ops[:],
                    zt_sb[pr][:, st * P : (st + 1) * P],
                    wo_sb[:, pr, nn * QB : (nn + 1) * QB],
                    start=(pr == 0),
                    stop=(pr == 1),
                )
            ot = ost.tile([P, QB], F32, name="ot", tag="ot")
            nc.vector.tensor_copy(ot[:], ops[:])
            nc.sync.dma_start(
                out=out_d[st * P : (st + 1) * P, nn * QB : (nn + 1) * QB],
                in_=ot[:],
            )

        def attn_kg_scores(p, qb, kg):
            """Scores + exp + diag-mask for one k-group (2 k-tiles)."""
            q0 = qb * QB
            offs = [(kg * 2 + j) * P - q0 for j in range(2)]
            deep = offs[0] >= 2 * P
            band = 0 <= offs[0] < 2 * P
            sA = sc.tile([P, 2, QB], F32, name="sA", tag="s")
            sB = sc.tile([P, 2, QB], F32, name="sB", tag="s")
            _chain([
                _mm(
                    nc,
                    stile[:, j, (offs[j] if deep else 0) : QB],
                    kt_sb[p][rows, (kg * 2 + j) * P : (kg * 2 + j + 1) * P],
                    qt_sb[p][rows, q0 + (offs[j] if deep else 0) : q0 + QB],
                    start=True,
                    stop=True,
                )
                for j in range(2)
                for rows, stile in ((slice(0, DH), sA), (slice(DH, P), sB))
            ])
            pA = ppool.tile([P, 2, QB], LO, name="pA", tag="pt")
            pB = ppool.tile([P, 2, QB], LO, name="pB", tag="pt")
            EXP = mybir.ActivationFunctionType.Exp
            escale = 0.125 / (WSCALE * WSCALE) if fp8 else 0.125
            if deep:
                # only the causally-live right part of each k-tile
                for px, sx in ((pA, sA), (pB, sB)):
                    for j in range(2):
                        c0 = offs[j]
                        nc.scalar.activation(
                            px[:, j, c0:QB], sx[:, j, c0:QB], EXP, scale=escale
                        )
                    # diagonal 128-blocks get the triangular mask
                    nc.vector.tensor_mul(
                        px[:, 0, offs[0] : offs[0] + P],
                        px[:, 0, offs[0] : offs[0] + P],
                        tri,
                    )
                    nc.vector.tensor_mul(
                        px[:, 1, offs[1] : offs[1] + P],
                        px[:, 1, offs[1] : offs[1] + P],
                        tri,
                    )
            else:
                # one merged exp per [128, 1024] tile
                nc.scalar.activation(pA[:], sA[:], EXP, scale=escale)
                nc.scalar.activation(pB[:], sB[:], EXP, scale=escale)
                if band:
                    # j=0 diag at cols 0:128 (tri); j=1 cols 0:256 get
                    # [zeros | tri] (left half is causally dead but was exp'd
                    # in the merged call; zeroing it makes the whole tile a
                    # valid summand for the per-kg denominator pre-add)
                    for px in (pA, pB):
                        nc.vector.tensor_mul(px[:, 0, 0:P], px[:, 0, 0:P], tri)
                        nc.vector.tensor_mul(
                            px[:, 1, 0 : 2 * P],
                            px[:, 1, 0 : 2 * P],
                            masks_sb[:, P : 3 * P],
                        )
            return pA, pB

        def attn_kg_pv(p, qb, kg, pA, pB, zps, dnb, nk):
            q0 = qb * QB
            for j in range(2):
                kt = kg * 2 + j
                c0 = max(kt * P - q0, 0)
                vA = v_all[:, kt, p * P : p * P + DH]
                vB = v_all[:, kt, p * P + DH : (p + 1) * P]
                st_, sp_ = (kt == 0), (kt == nk - 1)
                _chain([
                    _mm(
                        nc, zps[0:DH, c0:QB], vA, pA[:, j, c0:QB],
                        start=st_, stop=sp_, skip=True,
                    ),
                    _mm(
                        nc, zps[DH:P, c0:QB], vB, pB[:, j, c0:QB],
                        start=st_, stop=sp_, skip=True,
                    ),
                    _mm(
                        nc, dnb[0:DH, c0:QB], ones64[:], pA[:, j, c0:QB],
                        start=st_, stop=sp_, skip=True,
                    ),
                    _mm(
                        nc, dnb[DH:P, c0:QB], ones64[:], pB[:, j, c0:QB],
                        start=st_, stop=sp_, skip=True,
                    ),
                ])

        def attn_qb_tail(p, qb, zps, dnb):
            q0 = qb * QB
            bcs = bcpool.tile([P, QB], F32, name="bcs", tag="bcs")
            bcr = bcpool.tile([P, QB], F32, name="bcr", tag="bcr")
            nc.vector.reciprocal_approx_accurate(
                out=bcr[:], in_=dnb[:], scratch=bcs[:]
            )
            nc.vector.tensor_mul(zt_sb[p][:, q0 : q0 + QB], zps[:], bcr[:])

        # ---------------- emission schedule ----------------
        # pair-0 QK chunks, t-major K/Q pairs (fill the input-DMA shadow)
        for ch in range(NQB):
            qk_chunk_pair(0, ch)
        for st in range(4):
            v_unit(st)

        # filler units woven between attention k-groups to keep PE fed while
        # ScalarE works through the exp backlog. attention(0) has 20 k-groups
        # = exactly v(12) + pair-1 qk(8) fillers.
        fillers = [lambda st=st: v_unit(st) for st in range(4, NKT)]
        for ch in range(NQB):
            fillers.append(lambda ch=ch: qk_chunk(1, "wk", kt_sb[1], ch))
            fillers.append(lambda ch=ch: qk_chunk(1, "wq", qt_sb[1], ch))

        def pop_filler(n=1):
            for _ in range(n):
                if fillers:
                    fillers.pop(0)()

        def attention(p, qb):
            # software pipeline: PV runs one k-group behind scores, so the
            # exp of k-group kg has the whole window [scores kg+1, filler,
            # PV kg-1] (~2.2us of PE work) to complete before PV kg needs it
            nk = 4 * (qb + 1)
            zps = zp.tile([P, QB], F32, name="zps", tag="z")
            dnb = dp.tile([P, QB], F32, name="dnb", tag="d")
            prevs = []
            for kg in range(nk // 2):
                cur = attn_kg_scores(p, qb, kg)
                pop_filler()
                if len(prevs) == 2:
                    attn_kg_pv(p, qb, kg - 2, *prevs.pop(0), zps, dnb, nk)
                prevs.append(cur)
            for i, pv in enumerate(prevs):
                attn_kg_pv(p, qb, nk // 2 - len(prevs) + i, *pv, zps, dnb, nk)
            attn_qb_tail(p, qb, zps, dnb)

        for qb in range(NQB):
            attention(0, qb)
        pop_filler(len(fillers))  # any remaining pair-1 QK chunks

        # pair 1 with O-projection of the previous qb's seq tiles as filler
        for qb in range(NQB):
            if qb > 0:
                fillers.extend(
                    lambda st=st, nn=nn: o_unit(st, nn)
                    for st in range(4 * (qb - 1), 4 * qb)
                    for nn in range(2)
                )
            attention(1, qb)
        pop_filler(len(fillers))  # remaining O units for st 8-11
        for st in range(12, 16):
            for nn in range(2):
                o_unit(st, nn)

    nc.compile()
    _PROGRAM_CACHE[key] = nc
    return nc


def _round_fp32r(a):
    """Round fp32 array to nearest fp32r (drop 12 low mantissa bits)."""
    u = np.ascontiguousarray(a, dtype=np.float32).view(np.uint32)
    u = ((u + 0x800) & np.uint32(0xFFFFF000)).astype(np.uint32)
    return u.view(np.float32)


def make_in_maps(
    normalized_resid_pre, W_Q, W_K, W_V, W_O, b_Q, b_K, b_V, b_O,
    mm_dtype=MM_DTYPE, with_bias=False,
):
    """Shard + prearrange the full inputs into per-core input maps."""
    import ml_dtypes  # noqa: F401  (registers bfloat16 with numpy)

    fp8 = mm_dtype == "fp8"
    np_hi = np.dtype("bfloat16") if mm_dtype in ("bf16", "fp8") else np.float32
    np_lo = np.float32 if mm_dtype == "fp32" else np.dtype("bfloat16")
    np_p8 = np.dtype("float8_e4m3fn")
    rnd = _round_fp32r if mm_dtype == "mixed" else (lambda a: a)

    x = np.asarray(normalized_resid_pre, dtype=np.float32)
    W_Q = np.asarray(W_Q, dtype=np.float32)
    W_K = np.asarray(W_K, dtype=np.float32)
    W_V = np.asarray(W_V, dtype=np.float32)
    W_O = np.asarray(W_O, dtype=np.float32)

    def retile_rows(a):
        # [NDM*128, C] -> [128, NDM, C]
        c = a.shape[1]
        return np.ascontiguousarray(
            a.reshape(-1, P, c).transpose(1, 0, 2)
        )

    def retile_dr(a):
        # [NDM*128, C] -> [128, NDM/2, 2, C]: DoubleRow 256-contraction
        # blocks; (r, tp, e) holds input row tp*256 + 2r + e
        c = a.shape[1]
        return np.ascontiguousarray(
            a.reshape(-1, P, 2, c).transpose(1, 0, 2, 3)
        )

    retile_x = retile_dr if fp8 else retile_rows
    np_x = np_p8 if fp8 else np_hi
    xT = [
        retile_x(rnd(np.ascontiguousarray(x[b].T))).astype(np_x)
        for b in range(B)
    ]

    kp = np.arange(P)[:, None]
    qc = np.arange(P)[None, :]
    tri = (qc >= kp).astype(np.float32)
    masks = np.concatenate(
        [tri, np.zeros((P, P), dtype=np.float32), tri], axis=1
    ).astype(np_lo)

    in_maps = []
    for c in range(NCORES):
        b = c // (NCORES // B)
        heads = [HPC * (c % (NCORES // B)) + i for i in range(HPC)]
        ws = WSCALE if fp8 else 1.0
        wq = retile_x(
            rnd(ws * np.concatenate([W_Q[h] for h in heads], axis=1))
        ).astype(np_x)
        wk = retile_x(
            rnd(ws * np.concatenate([W_K[h] for h in heads], axis=1))
        ).astype(np_x)
        wv = retile_x(
            rnd(ws * np.concatenate([W_V[h] for h in heads], axis=1))
        ).astype(np_x)
        wo = retile_rows(
            rnd(np.concatenate([W_O[h] for h in heads], axis=0) / ws)
        ).astype(np_hi)
        m = {
            "xT": xT[b], "wq": wq, "wk": wk, "wv": wv, "wo": wo,
            "masks": masks,
        }
        if with_bias:
            # fp8 path scales Q/K/V by WSCALE on-device; biases must match
            b_Qa = ws * np.asarray(b_Q, dtype=np.float32)
            b_Ka = ws * np.asarray(b_K, dtype=np.float32)
            b_Va = ws * np.asarray(b_V, dtype=np.float32)
            m["bq"] = np.stack(
                [
                    np.concatenate([b_Qa[heads[0]], b_Qa[heads[1]]]),
                    np.concatenate([b_Qa[heads[2]], b_Qa[heads[3]]]),
                ]
            ).astype(np.float32)
            m["bk"] = np.stack(
                [
                    np.concatenate([b_Ka[heads[0]], b_Ka[heads[1]]]),
                    np.concatenate([b_Ka[heads[2]], b_Ka[heads[3]]]),
                ]
            ).astype(np.float32)
            m["bv"] = np.tile(
                np.concatenate([b_Va[h] for h in heads])[None, :], (P, 1)
            ).astype(np.float32)
        in_maps.append(m)
    return in_maps


def kernel(normalized_resid_pre, W_Q, W_K, W_V, W_O, b_Q, b_K, b_V, b_O):
    global LAST_RESULTS
    b_Q = np.asarray(b_Q, dtype=np.float32)
    b_K = np.asarray(b_K, dtype=np.float32)
    b_V = np.asarray(b_V, dtype=np.float32)
    b_O = np.asarray(b_O, dtype=np.float32)
    with_bias = bool(
        np.abs(b_Q).max() or np.abs(b_K).max() or np.abs(b_V).max()
    )
    nc = build_program(MM_DTYPE, with_bias)
    in_maps = make_in_maps(
        normalized_resid_pre, W_Q, W_K, W_V, W_O, b_Q, b_K, b_V, b_O,
        MM_DTYPE, with_bias,
    )
    trace = os.environ.get("ATTN_TRACE", "0") == "1"
    res = run_bass_kernel_spmd(nc, in_maps, list(range(NCORES)), trace=trace)
    LAST_RESULTS = res

    parts = [
        np.asarray(res.results[c]["out"], dtype=np.float64)
        for c in range(NCORES)
    ]
    npc = NCORES // B  # cores per batch
    out = np.stack(
        [sum(parts[b * npc : (b + 1) * npc]) + b_O for b in range(B)]
    )
    return out.astype(np.float32)
